# revision 53
# baseline (speedup 1.0000x reference)
"""Trainium2 Bass kernel for nn_MAB_2121713844542 (dense transformer block).

Strategy: data-parallel over batch B=32 across 8 cores (4 batches/core).
All activations kept in transposed layout [feature, seq] so every matmul
consumes operands directly (contraction dim on partitions) with zero
on-device transposes: Q/K/p are transposed on host before upload.

v6 (487us -> ~308us): bf16 matmuls everywhere (PSUM accumulates fp32;
the headline gate absmax/max|expected| < 2e-2 leaves plenty of room).
Softmax divide: denominators (ones-column rows of the AV PSUM) are
gathered as bf16 to 32-aligned rows of a zeroed staging tile (DVE
partition bases must be 32-aligned), PE-broadcast to all 128 partitions
via the EB matmul, evacuated to SBUF by ScalarE, and inverted with ONE
single-instruction reciprocal_approx_fast per quad (the bit-trick
custom DVE op; plain nc.vector.reciprocal costs 3.4us per [1,512]
row).  LN gain/beta fold into PE rank-1 broadcasts (bcAg = g (x) rstd,
bcD = g (x) mean*rstd - beta (x) 1) so the apply is 2 DVE ops per
128-feature tile; rstd = exp(-0.5*ln(var+eps)) keeps ACT on one table
set pair.  FFN-2 evacuation+bias+residual fuse into one
scalar_tensor_tensor.  The batch loop is software-pipelined at phase
granularity (LN stats hoisted before the next batch's projections,
quad-1 scores covering quad-0's divide chain, LN0 sum matmuls deferred
behind the next iteration's queue head, the tail's second-to-last LN
hidden under the last FFN) so the in-order PE queue always holds
independent matmuls while DVE/ACT chains run — otherwise the PE idles
and the HAM clock gate re-throttles it to 1.2 GHz.
"""

import os
import numpy as np
import ml_dtypes

import concourse.bass as bass
import concourse.mybir as mybir
import concourse.tile as tile
from concourse import bacc
from concourse.bass_utils import run_bass_kernel_spmd

TAP = os.environ.get("BASS_TAP", "")

B, S, D, H, DH, DFF = 32, 512, 256, 8, 32, 2048
NCORES = 8
BL = B // NCORES
P = 128
DT = D // P     # 2 feature tiles
FT = DFF // P   # 16 ffn tiles
ST = S // P     # 4 seq tiles
f32 = mybir.dt.float32
bf16 = mybir.dt.bfloat16
AF = mybir.ActivationFunctionType
ALU = mybir.AluOpType
EPS = 1e-5
VW = 33 * H + 33   # VhA free width (33 per head + tail for 64-wide lhsT)


def build_nc():
    nc = bacc.Bacc("TRN2", target_bir_lowering=False, debug=False,
                   num_devices=NCORES)

    QT = nc.dram_tensor("QT", (BL, P, DT, S), bf16, kind="ExternalInput")
    KT = nc.dram_tensor("KT", (BL, P, DT, S), bf16, kind="ExternalInput")
    pT = nc.dram_tensor("pT", (BL, 4, S), bf16, kind="ExternalInput")
    Wq = nc.dram_tensor("Wq", (P, DT, D), bf16, kind="ExternalInput")
    Wk = nc.dram_tensor("Wk", (P, DT, D), bf16, kind="ExternalInput")
    Wv = nc.dram_tensor("Wv", (P, DT, D), bf16, kind="ExternalInput")
    Wp = nc.dram_tensor("Wp", (4, D), bf16, kind="ExternalInput")
    W1 = nc.dram_tensor("W1", (P, DT, DFF), bf16, kind="ExternalInput")
    W2 = nc.dram_tensor("W2", (P, FT, D), bf16, kind="ExternalInput")
    EB = nc.dram_tensor("EB", (P, P), bf16, kind="ExternalInput")
    gr0 = nc.dram_tensor("gr0", (1, D), bf16, kind="ExternalInput")
    nb0 = nc.dram_tensor("nb0", (1, D), bf16, kind="ExternalInput")
    gr1 = nc.dram_tensor("gr1", (1, D), bf16, kind="ExternalInput")
    nb1 = nc.dram_tensor("nb1", (1, D), bf16, kind="ExternalInput")
    bq = nc.dram_tensor("bq", (P, DT), f32, kind="ExternalInput")
    bk = nc.dram_tensor("bk", (P, DT), f32, kind="ExternalInput")
    bp = nc.dram_tensor("bp", (P, DT), f32, kind="ExternalInput")
    bv = nc.dram_tensor("bv", (P, D), f32, kind="ExternalInput")
    b1 = nc.dram_tensor("b1", (P, FT), f32, kind="ExternalInput")
    b2 = nc.dram_tensor("b2", (P, DT), f32, kind="ExternalInput")
    outT = nc.dram_tensor("outT", (BL, P, DT, S), f32, kind="ExternalOutput")

    with tile.TileContext(nc) as tc:
        with (
            tc.tile_pool(name="singles", bufs=1) as singles,
            tc.tile_pool(name="inbuf", bufs=2) as inbuf,
            tc.tile_pool(name="proj", bufs=2) as proj,
            tc.tile_pool(name="attn", bufs=2) as attn,
            tc.tile_pool(name="ffn", bufs=2) as ffn,
            tc.tile_pool(name="small", bufs=2) as small,
            tc.tile_pool(name="stats", bufs=2) as stats,
            tc.tile_pool(name="outp", bufs=2) as outp,
            tc.tile_pool(name="ps_mm", bufs=3, space="PSUM") as ps_mm,
            tc.tile_pool(name="ps_av", bufs=3, space="PSUM") as ps_av,
            tc.tile_pool(name="ps_st", bufs=2, space="PSUM") as ps_st,
        ):
            # ---- one-time constants / weights ----
            def load(dram, shape):
                t = singles.tile(list(shape), dram.dtype, name="w_" + dram.name)
                nc.sync.dma_start(t, dram[tuple(slice(None) for _ in shape)])
                return t

            Wq_sb = load(Wq, (P, DT, D))
            EB_sb = load(EB, (P, P))
            Wk_sb = load(Wk, (P, DT, D))
            Wv_sb = load(Wv, (P, DT, D))
            Wp_sb = load(Wp, (4, D))
            W1_sb = load(W1, (P, DT, DFF))
            W2_sb = load(W2, (P, FT, D))
            gr0_sb = load(gr0, (1, D))
            nb0_sb = load(nb0, (1, D))
            gr1_sb = load(gr1, (1, D))
            nb1_sb = load(nb1, (1, D))

            def loadj(dram, shape):
                # stage through a DVE copy so TensorScalar consumers get a
                # same-engine dep instead of a DMA semaphore wait
                st = load(dram, shape)
                t = singles.tile(list(shape), f32, name="j_" + dram.name)
                nc.vector.tensor_copy(t, st)
                return t

            bq_sb = loadj(bq, (P, DT))
            bk_sb = loadj(bk, (P, DT))
            bp_sb = loadj(bp, (P, DT))
            b1_sb = loadj(b1, (P, FT))
            b2_sb = loadj(b2, (P, DT))
            bv_bc = loadj(bv, (P, D))

            eps1 = singles.tile([1, 1], f32)
            nc.vector.memset(eps1, EPS)
            onesC = singles.tile([P, 1], bf16)   # lhsT for partition sums
            nc.vector.memset(onesC, 1.0)
            ones_row = singles.tile([1, S], bf16)
            nc.vector.memset(ones_row, 1.0)
            # softmax denominators live at rows 32*h4 (DVE partition bases
            # must be 32-aligned); other rows stay zero forever so the EB
            # broadcast matmul never sees garbage.
            dent = [singles.tile([P, S], bf16, name=f"dent{q}")
                    for q in range(2)]
            nc.vector.memset(dent[0], 0.0)
            nc.vector.memset(dent[1], 0.0)

            # ---- phase A: input DMA + projections ----
            def phase_A(b):
                QT_sb = inbuf.tile([P, DT, S], bf16, tag="qt")
                nc.sync.dma_start(QT_sb, QT[b])
                KT_sb = inbuf.tile([P, DT, S], bf16, tag="kt")
                nc.sync.dma_start(KT_sb, KT[b])
                pT_sb = small.tile([4, S], bf16, tag="pt")
                nc.sync.dma_start(pT_sb, pT[b])

                QhT = proj.tile([P, DT, S], bf16, tag="qh")
                KhT = proj.tile([P, DT, S], bf16, tag="kh")
                PhT = proj.tile([P, DT, S], bf16, tag="ph")
                for t in range(DT):
                    ps = ps_mm.tile([P, S], f32, tag="ps")
                    for kt in range(DT):
                        nc.tensor.matmul(
                            ps, Wq_sb[:, kt, t * P : (t + 1) * P],
                            QT_sb[:, kt, :],
                            start=(kt == 0), stop=(kt == DT - 1))
                    nc.vector.tensor_tensor(QhT[:, t, :], ps, bq_sb[:, t : t + 1].to_broadcast((P, S)), ALU.add)
                    ps = ps_mm.tile([P, S], f32, tag="ps")
                    for kt in range(DT):
                        nc.tensor.matmul(
                            ps, Wk_sb[:, kt, t * P : (t + 1) * P],
                            KT_sb[:, kt, :],
                            start=(kt == 0), stop=(kt == DT - 1))
                    nc.vector.tensor_tensor(KhT[:, t, :], ps, bk_sb[:, t : t + 1].to_broadcast((P, S)), ALU.add)
                    ps = ps_mm.tile([P, S], f32, tag="ps")
                    nc.tensor.matmul(ps, Wp_sb[:, t * P : (t + 1) * P],
                                     pT_sb, start=True, stop=True)
                    nc.vector.tensor_tensor(PhT[:, t, :], ps, bp_sb[:, t : t + 1].to_broadcast((P, S)), ALU.add)

                # Vh natural layout with a ones column per head: [P, ST, VW]
                VhA = proj.tile([P, ST, VW], bf16, tag="vh")
                VhA_h = VhA[:, :, 0 : 33 * (H + 1)].rearrange(
                    "p s (h c) -> p s h c", c=33)
                nc.vector.tensor_copy(
                    VhA_h[:, :, 0:H, 32:33],
                    onesC[:, 0:1].to_broadcast((P, ST, H, 1)))
                for st in range(ST):
                    ps = ps_mm.tile([P, S], f32, tag="ps")
                    for kt in range(DT):
                        nc.tensor.matmul(
                            ps[:, :D], KT_sb[:, kt, st * P : (st + 1) * P],
                            Wv_sb[:, kt, :],
                            start=(kt == 0), stop=(kt == DT - 1))
                    nc.vector.tensor_tensor(
                        VhA_h[:, st, 0:H, 0:32],
                        ps[:, 0:D].rearrange("p (h c) -> p h c", c=32),
                        bv_bc.rearrange("p (h c) -> p h c", c=32),
                        ALU.add)
                return dict(QhT=QhT, KhT=KhT, PhT=PhT, VhA=VhA)

            # ---- phase S: attention (scores, exp, AV, softmax-divide,
            #      residual, LN0 partial sums) ----
            def phase_S(b, st8, head_only=False):
                QhT, KhT, PhT, VhA = (st8["QhT"], st8["KhT"], st8["PhT"],
                                      st8["VhA"])
                if head_only:
                    OT = proj.tile([P, DT, S], bf16, tag="ot")
                    x2 = small.tile([P, DT, S], bf16, tag="x2")
                    st8["OT"] = OT
                    st8["_x2"] = x2
                    sxx = ps_st.tile([33, S], f32, tag="sx", name="sxx")
                    st8["sxx"] = sxx
                else:
                    OT = st8["OT"]
                    x2 = st8["_x2"]
                    sxx = st8["sxx"]

                def quad_scores_av(quad):
                    expS = [attn.tile([P, ST, S], bf16, tag=f"expS{i}",
                                      name=f"expS{i}")
                            for i in range(4)]
                    for kt in range(ST):
                        for hp in range(2):        # 2 heads per PSUM pass
                            pss = {}
                            for h4 in (2 * hp, 2 * hp + 1):
                                base = 32 * h4
                                ps = ps_mm.tile([P, S], f32, tag="ps")
                                pss[h4] = ps
                                nc.tensor.matmul(
                                    ps,
                                    KhT[base : base + 32, quad, kt * P : (kt + 1) * P],
                                    QhT[base : base + 32, quad, :],
                                    start=True, stop=False,
                                    tile_position=(base, 0))
                            for h4 in (2 * hp, 2 * hp + 1):
                                base = 32 * h4
                                nc.tensor.matmul(
                                    pss[h4],
                                    PhT[base : base + 32, quad, kt * P : (kt + 1) * P],
                                    PhT[base : base + 32, quad, :],
                                    start=False, stop=True,
                                    tile_position=(base, 0))
                            for h4 in (2 * hp, 2 * hp + 1):
                                nc.scalar.activation(expS[h4][:, kt, :],
                                                     pss[h4], AF.Exp)
                    avAB = [ps_av.tile([P, S], f32, tag="av", name="avA"),
                            ps_av.tile([P, S], f32, tag="av", name="avB")]
                    for h4 in range(4):
                        h = 4 * quad + h4
                        tgt = avAB[h4 // 2]
                        rb = 64 * (h4 % 2)
                        for kt in range(ST):
                            nc.tensor.matmul(
                                tgt[rb : rb + 64, :],
                                VhA[:, kt, 33 * h : 33 * h + 64],
                                expS[h4][:, kt, :],
                                start=(kt == 0), stop=(kt == ST - 1))
                        # gather denom (bf16 cast) to its 32-aligned row
                        nc.vector.tensor_copy(
                            dent[quad][32 * h4 : 32 * h4 + 1, :],
                            tgt[rb + 32 : rb + 33, :])
                    return expS, avAB

                def quad_epilogue_a(quad, avAB):
                    bcd = ps_av.tile([P, S], f32, tag="av", name="bcd")
                    nc.tensor.matmul(bcd, EB_sb, dent[quad],
                                     start=True, stop=True)
                    bcs = small.tile([P, S], f32, tag="bcs")
                    nc.scalar.activation(bcs, bcd, AF.Copy)
                    rec = small.tile([P, S], f32, tag="rec")
                    nc.vector.reciprocal_approx_fast(rec, bcs)
                    for h4 in range(4):
                        tgt = avAB[h4 // 2]
                        rb = 64 * (h4 % 2)
                        sl = slice(32 * h4, 32 * h4 + 32)
                        nc.vector.tensor_mul(OT[sl, quad, :],
                                             tgt[rb : rb + 32, :], rec[sl, :])
                        nc.vector.tensor_add(OT[sl, quad, :], OT[sl, quad, :],
                                             QhT[sl, quad, :])
                    nc.vector.tensor_mul(x2[:, quad, :], OT[:, quad, :],
                                         OT[:, quad, :])

                def quad_epilogue_b(quad):
                    # LN0 partial-sum matmuls, issued late so the PE is not
                    # head-of-line blocked on the quad's DVE chain
                    nc.tensor.matmul(sxx[0:1, :], onesC, OT[:, quad, :],
                                     start=(quad == 0), stop=(quad == 1),
                                     skip_group_check=True)
                    nc.tensor.matmul(sxx[32:33, :], onesC, x2[:, quad, :],
                                     start=(quad == 0), stop=(quad == 1),
                                     tile_position=(0, 32),
                                     skip_group_check=True)

                if head_only:
                    st8["_sav0"] = quad_scores_av(0)
                    return
                _, av0 = st8["_sav0"]
                quad_epilogue_a(0, av0)
                _, av1 = quad_scores_av(1)  # PE work hiding quad0's chain
                quad_epilogue_b(0)
                quad_epilogue_a(1, av1)
                st8["_epb1"] = lambda: quad_epilogue_b(1)

            # natural_log_exp_and_others: has BOTH ln and exp, so one load
            # covers the whole LN chain AND the following scores exps. The
            # compiler's first-match set choice would otherwise emit two
            # serial 1.28us loads (natural_log, then exp_and_others) inside
            # every LN stats chain. Pre-placing the load also lets it run
            # early (no data deps) under the DVE mean/var ops.
            NL_EXP_SET = 6

            def load_nlexp():
                nc.scalar.add_instruction(mybir.InstLoadActFuncSet(
                    name=nc.get_next_instruction_name(), ins=[], outs=[],
                    act_func_set_id=NL_EXP_SET))

            # ---- layer-norm: stats chain (DVE/ACT only), then
            #      fused-broadcast apply (PE rank-1 bcasts + 2 DVE ops/t).
            #      Split so the stats chains of two LNs can be hoisted
            #      before the next batch's projection matmuls.
            def ln_stats(sxx):
                mean = stats.tile([1, S], f32, tag="m")
                nc.vector.tensor_scalar_mul(mean, sxx[0:1, :], 1.0 / D)
                m2 = stats.tile([1, S], f32, tag="m2")
                nc.vector.tensor_mul(m2, mean, mean)
                var = stats.tile([1, S], f32, tag="v")
                nc.vector.scalar_tensor_tensor(
                    var, sxx[32:33, :], 1.0 / D, m2, ALU.mult, ALU.subtract)
                lnv = stats.tile([1, S], f32, tag="ln")
                nc.scalar.activation(lnv, var, AF.Ln, bias=eps1)
                A = stats.tile([1, S], bf16, tag="A")
                nc.scalar.activation(A, lnv, AF.Exp, scale=-0.5)
                C = stats.tile([1, S], bf16, tag="C")
                nc.vector.tensor_mul(C, mean, A)
                return A, C

            def ln_apply(AC, x_sb, grow, nbrow, out_sb):
                A, C = AC
                for t in range(DT):
                    bcAg = ps_av.tile([P, S], f32, tag="av", name="bcAg")
                    nc.tensor.matmul(bcAg, grow[0:1, t * P:(t + 1) * P], A,
                                     start=True, stop=True)
                    bcD = ps_av.tile([P, S], f32, tag="av", name="bcD")
                    nc.tensor.matmul(bcD, grow[0:1, t * P:(t + 1) * P], C,
                                     start=True, stop=False)
                    nc.tensor.matmul(bcD, nbrow[0:1, t * P:(t + 1) * P],
                                     ones_row, start=False, stop=True)
                    nc.vector.tensor_mul(out_sb[:, t, :], x_sb[:, t, :], bcAg)
                    nc.vector.tensor_sub(out_sb[:, t, :], out_sb[:, t, :], bcD)

            # ---- phase E: LN0 ----
            def phase_E_stats(b, st8):
                st8["AC0"] = ln_stats(st8["sxx"])

            def phase_E_apply(b, st8):
                LN1 = proj.tile([P, DT, S], bf16, tag="ln1")
                ln_apply(st8["AC0"], st8["OT"], gr0_sb, nb0_sb, LN1)
                st8["LN1"] = LN1

            # ---- phase L1: FFN ----
            def phase_L1(b, st8):
                LN1 = st8["LN1"]
                G = ffn.tile([P, FT, S], bf16, tag="g")
                for ft in range(FT):
                    ps = ps_mm.tile([P, S], f32, tag="ps")
                    for t in range(DT):
                        nc.tensor.matmul(
                            ps, W1_sb[:, t, ft * P : (ft + 1) * P],
                            LN1[:, t, :],
                            start=(t == 0), stop=(t == DT - 1))
                    nc.scalar.activation(G[:, ft, :], ps, AF.Gelu,
                                         bias=b1_sb[:, ft : ft + 1])
                Z = small.tile([P, DT, S], bf16, tag="z")
                x2b = small.tile([P, DT, S], bf16, tag="x2b")
                sxx2 = ps_st.tile([33, S], f32, tag="sx", name="sxx2")
                for t in range(DT):
                    ps = ps_mm.tile([P, S], f32, tag="ps")
                    for ft in range(FT):
                        nc.tensor.matmul(
                            ps, W2_sb[:, ft, t * P : (t + 1) * P],
                            G[:, ft, :],
                            start=(ft == 0), stop=(ft == FT - 1))
                    nc.vector.scalar_tensor_tensor(
                        Z[:, t, :], ps, b2_sb[:, t : t + 1], LN1[:, t, :],
                        ALU.add, ALU.add)
                    nc.vector.tensor_mul(x2b[:, t, :], Z[:, t, :], Z[:, t, :])
                    nc.tensor.matmul(sxx2[0:1, :], onesC, Z[:, t, :],
                                     start=(t == 0), stop=(t == 1),
                                     skip_group_check=True)
                    nc.tensor.matmul(sxx2[32:33, :], onesC, x2b[:, t, :],
                                     start=(t == 0), stop=(t == 1),
                                     tile_position=(0, 32),
                                     skip_group_check=True)
                st8["Z"] = Z
                st8["sxx2"] = sxx2

            # ---- phase L2: final LN + output ----
            def phase_L2_stats(b, st8):
                st8["AC1"] = ln_stats(st8["sxx2"])

            def phase_L2_apply(b, st8):
                OUT = outp.tile([P, DT, S], f32, tag="out")
                ln_apply(st8["AC1"], st8["Z"], gr1_sb, nb1_sb, OUT)
                if TAP:
                    tap_src = {"ot": st8["OT"],
                               "ln1": st8["LN1"], "z": st8["Z"]}[TAP]
                    nc.vector.tensor_copy(OUT, tap_src)
                nc.sync.dma_start(outT[b], OUT)

            # ---- software-pipelined batch loop ----
            # issue order per step: A(i+1), L2(i-1), E(i), S(i+1), L1(i) —
            # the in-order PE queue always holds independent matmuls while
            # this batch's DVE/ACT chains run.
            sts = {0: phase_A(0)}
            phase_S(0, sts[0], head_only=True)
            for i in range(BL):
                last = (i == BL - 1)
                # A's projection evacuations go FIRST on the DVE queue so
                # the next batch's score matmuls are ready before the PE
                # exhausts its cover; the LN stats chains queue after them.
                if not last:
                    sts[i + 1] = phase_A(i + 1)
                if i == 0:
                    # batch 0's attention tail here, so its epilogue chains
                    # are covered by A(1)'s and S_head(1)'s matmuls (S_head
                    # must come after it: ps_av ring ordering)
                    phase_S(0, sts[0])
                if not last:
                    phase_S(i + 1, sts[i + 1], head_only=True)
                if "_epb1" in sts[i]:
                    sts[i].pop("_epb1")()  # finish batch i's LN0 sums
                if i - 1 >= 0 and not last:
                    phase_L2_stats(i - 1, sts[i - 1])
                phase_E_stats(i, sts[i])
                if i - 1 >= 0 and not last:
                    phase_L2_apply(i - 1, sts[i - 1])
                    del sts[i - 1]
                phase_E_apply(i, sts[i])
                if not last:
                    phase_S(i + 1, sts[i + 1])
                phase_L1(i, sts[i])
            # tail: batch BL-2's final LN hides under batch BL-1's FFN
            phase_L2_stats(BL - 2, sts[BL - 2])
            phase_L2_apply(BL - 2, sts[BL - 2])
            phase_L2_stats(BL - 1, sts[BL - 1])
            phase_L2_apply(BL - 1, sts[BL - 1])

    nc.finalize()
    return nc


_NC = None


def kernel(Q, K, p, Wq, bq, Wk, bk, Wv, bv, Wp, bp, g0, beta0, W1, b1, W2, b2,
           g1, beta1):
    global _NC
    if _NC is None:
        _NC = build_nc()

    f = np.float32
    bf = ml_dtypes.bfloat16

    def feat_tiles(x):  # [B, S, D] -> [B, P, DT, S] bf16
        x = np.asarray(x, f).transpose(0, 2, 1).reshape(-1, DT, P, S)
        return np.ascontiguousarray(x.transpose(0, 2, 1, 3)).astype(bf)

    def pp(vec, n):  # [n*P] -> [P, n]
        return np.ascontiguousarray(np.asarray(vec, f).reshape(n, P).T)

    def wmat(w, n, m):  # [n*P, m] -> [P, n, m] bf16
        w = np.asarray(w, f).reshape(n, P, m)
        return np.ascontiguousarray(w.transpose(1, 0, 2)).astype(bf)

    def row(vec):  # [D] -> [1, D] bf16
        return np.asarray(vec, f).reshape(1, D).astype(bf)

    QTf = feat_tiles(Q)
    KTf = feat_tiles(K)
    # pad p's 3-channel dim to 4 (zero row) and pre-scale the PE projection
    # by 1/4 so PhPh^T carries the 1/sqrt(DV)=1/16 score scaling.
    pTf = np.zeros((B, 4, S), f)
    pTf[:, :3, :] = np.transpose(np.asarray(p, f), (0, 2, 1))
    pTf = pTf.astype(bf)
    Wp4 = np.zeros((4, D), f)
    Wp4[:3] = np.asarray(Wp, f) * 0.25
    bp4 = np.asarray(bp, f) * 0.25  # kernel consumes bp already scaled

    EBm = np.zeros((P, P), f)
    for h4 in range(4):
        EBm[32 * h4, 32 * h4 : 32 * h4 + 32] = 1.0

    shared = {
        "EB": EBm.astype(bf),
        "Wq": wmat(Wq, DT, D), "Wk": wmat(Wk, DT, D), "Wv": wmat(Wv, DT, D),
        "Wp": Wp4.astype(bf),
        "W1": wmat(W1, DT, DFF), "W2": wmat(W2, FT, D),
        "bq": pp(bq, DT), "bk": pp(bk, DT),
        "bv": np.ascontiguousarray(np.broadcast_to(np.asarray(bv, f), (P, D))),
        "bp": pp(bp4, DT),
        "b1": pp(b1, FT), "b2": pp(b2, DT),
        "gr0": row(g0), "nb0": row(-np.asarray(beta0, f)),
        "gr1": row(g1), "nb1": row(-np.asarray(beta1, f)),
    }
    in_maps = []
    for c in range(NCORES):
        m = dict(shared)
        m["QT"] = np.ascontiguousarray(QTf[c * BL : (c + 1) * BL])
        m["KT"] = np.ascontiguousarray(KTf[c * BL : (c + 1) * BL])
        m["pT"] = np.ascontiguousarray(pTf[c * BL : (c + 1) * BL])
        in_maps.append(m)

    trace = bool(os.environ.get("BASS_TRACE"))
    res = run_bass_kernel_spmd(_NC, in_maps, core_ids=list(range(NCORES)),
                               trace=trace)
    kernel._LAST = res
    outs = [res.results[c]["outT"] for c in range(NCORES)]
    full = np.concatenate(outs, axis=0)  # [B, P, DT, S]
    full = full.transpose(0, 2, 1, 3).reshape(B, D, S)  # [B, D, S]
    return np.ascontiguousarray(full.transpose(0, 2, 1))
